# revision 25
# baseline (speedup 1.0000x reference)
"""GAT message-passing kernel for 8 trn2 NeuronCores (v2).

Math (reference, per t):
    Wx = x @ W;  s1 = Wx@a1/sqrt(2D);  s2 = Wx@a2/sqrt(2D)
    weight = softmax_src(lrelu(s1[src] + s2[dest]));  agg = lrelu(weight @ Wx)
    out = x - agg

Identities used (uniform "relu-form"):
    exp(lrelu(s1+s2)) with per-dest rescale by exp(-s2) gives
        et[src,dest] = F1[src] * max(t[src], r[dest])
    with t = exp(0.99 s1), F1 = exp(0.01 s1), r = exp(-0.99 s2).
    Folding F1 into the matmul rhs (wxp = [F1*Wx | F1]) and writing
        max(t, r) = t + relu(r - t)
    makes the O(N^2) score tile a single-input op on ANY of DVE /
    Scalar(ACT) / GpSimd:  relu(r_b + (-t[src]))  — so all three engines
    build tiles concurrently.  The constant part contributes
        c[f] = sum_src t[src] * wxp[src, f]
    per accumulator row, injected exactly via one rank-1 matmul per
    accumulator (last matmul of the PSUM accumulation group).
    Denominator rides as wxp's 129th column (= F1).

Sharding: 8 cores = 4 t-slices x 2 N-halves; core aggregates over all
4096 src for its (t, 2048 dest).  Score tiles are built double-width
([128, 1024] = 2 q-chunks) once and consumed by 2 passes.
"""

import sys

if "/opt/trn_rl_repo" not in sys.path:
    sys.path.insert(0, "/opt/trn_rl_repo")

import numpy as np

N, T, D = 4096, 4, 128
P = 128
HALF = N // 2            # 2048 dest nodes per core
MT = N // P              # 32 src tiles
NT = HALF // P           # 16 dest chunks
SCALE_INV = 1.0 / 16.0   # 1/sqrt(2*128)
PG = 3                   # proj mts per PSUM bank ([128, 3*129] f32 = 1548B)
NPG = (MT + PG - 1) // PG

_CACHE = {}


def _tile_assignment():
    """Weighted round-robin engine assignment for the 64 double-tile builds.

    DVE max-form is ~426ns/tile (4x mode), ACT relu-form ~1131ns/tile, but
    during pass 0 the DVE also carries the wxp/s1a prologue copies — so
    pair 0 leans a bit more on ACT than pair 1 would suggest.
    """

    def rr(n, w):
        cnt = {e: 0 for e in w}
        out = []
        for _ in range(n):
            e = min(w, key=lambda k: (cnt[k] + 1) / w[k])
            cnt[e] += 1
            out.append(e)
        return out

    return rr(MT, {"d": 24.0, "a": 8.0}) + rr(MT, {"d": 22.0, "a": 10.0})


def _build():
    import concourse.mybir as mybir
    from concourse import bacc
    from concourse.tile import TileContext

    f32 = mybir.dt.float32
    bf16 = mybir.dt.bfloat16
    Alu = mybir.AluOpType
    Act = mybir.ActivationFunctionType

    nc = bacc.Bacc()
    prm_d = nc.declare_dram_parameter("prm", [P, 2 * D + 2], bf16, isOutput=False)
    xt_d = nc.declare_dram_parameter("xt", [P, N], bf16, isOutput=False)
    xn_d = nc.declare_dram_parameter("xn", [P, NT * D], f32, isOutput=False)
    out = nc.declare_dram_parameter("out", [P, NT * D], f32, isOutput=True)

    assign = _tile_assignment()  # 64 double tiles: [pair01 mt0..31, pair23 mt0..31]

    with TileContext(nc) as tc:
        with (
            tc.tile_pool(name="const", bufs=1) as cpool,
            tc.tile_pool(name="fpool", bufs=4) as fpool,
            tc.tile_pool(name="opool", bufs=4) as opool,
        ):
            # ---------------- input DMAs ----------------
            # first xt chunk small so projections start early; prm on the
            # other ring so it doesn't delay xt chunk 0
            xt = cpool.tile([P, N], bf16)
            prm = cpool.tile([P, 2 * D + 2], bf16)
            nc.sync.dma_start(xt[:, 0:512], xt_d[:, 0:512])
            nc.scalar.dma_start(prm[:, :], prm_d[:, :])
            nc.sync.dma_start(xt[:, 512 : 1536], xt_d[:, 512:1536])
            nc.scalar.dma_start(xt[:, 1536:2560], xt_d[:, 1536:2560])
            nc.sync.dma_start(xt[:, 2560:4096], xt_d[:, 2560:4096])
            xn_sb = cpool.tile([P, NT * D], f32)
            nc.scalar.dma_start(xn_sb[:, :], xn_d[:, :])
            W_sb = prm[:, 0:D]
            WT_sb = prm[:, D : 2 * D]
            av_sb = prm[:, 2 * D : 2 * D + 2]

            # persistent SBUF state
            wproj = cpool.tile([P, D + 1], bf16)       # [W | w1s]
            w2b = cpool.tile([P, P], bf16)
            r_b = cpool.tile([P, HALF], bf16)
            s1a = cpool.tile([P, MT], f32)
            t_a = cpool.tile([P, MT], f32)
            nt_a = cpool.tile([P, MT], f32)            # -t
            tb_a = cpool.tile([P, MT], bf16)           # t in bf16 (cvec lhsT)
            F1a = cpool.tile([P, MT], f32)
            wxp = cpool.tile([P, MT * (D + 1)], bf16)  # [F1*Wx | F1] per mt
            cvec_sb = [
                cpool.tile([1, D + 1], bf16, name=f"cvec{pr}") for pr in range(2)
            ]
            ones_1p = cpool.tile([1, P], bf16)
            nc.vector.memset(ones_1p[:, :], 1.0)
            dt = [
                cpool.tile([P, 1024], bf16, name=f"dt{i}") for i in range(2 * MT)
            ]

            with tc.tile_pool(name="ppsum", bufs=4, space="PSUM") as ppool, \
                 tc.tile_pool(name="rbsum", bufs=2, space="PSUM") as rpool, \
                 tc.tile_pool(name="cvsum", bufs=1, space="PSUM") as vpool:
                # wproj = [W | (W@a1)/16];  w2col = (W@a2)/16
                nc.vector.tensor_copy(wproj[:, 0:D], W_sb)
                w_ps = vpool.tile([P, 2], f32, tag="wps", name="w_ps")
                nc.tensor.matmul(w_ps[:, :], WT_sb, av_sb, start=True, stop=True)
                nc.scalar.activation(
                    wproj[:, D : D + 1], w_ps[:, 0:1], Act.Copy, scale=SCALE_INV
                )
                sc2 = cpool.tile([P, 1], f32)
                nc.scalar.activation(sc2[:, :], w_ps[:, 1:2], Act.Copy, scale=SCALE_INV)
                # w2b[p, f] = w2col[p] (broadcast along free dim)
                nc.vector.tensor_scalar(
                    w2b[:, :], W_sb, 0.0, sc2[:, :], Alu.mult, Alu.add
                )

                # r_b[p, n] = exp(-0.99 * s2[n]) replicated over partitions
                for q in range(4):
                    rb_ps = rpool.tile([P, 512], f32, tag="rb", name="rb_ps")
                    nc.tensor.matmul(
                        rb_ps[:, :],
                        w2b[:, :],
                        xt[:, q * 512 : (q + 1) * 512],
                        start=True,
                        stop=True,
                    )
                    nc.scalar.activation(
                        r_b[:, q * 512 : (q + 1) * 512], rb_ps[:, :], Act.Exp,
                        scale=-0.99,
                    )

                # ---- projection groups: p_ps = [Wx | s1] per mt ----
                for g in range(NPG):
                    mts = list(range(g * PG, min((g + 1) * PG, MT)))
                    w = len(mts)
                    p_ps = ppool.tile([P, w * (D + 1)], f32, tag="pj", name="p_ps")
                    for i, mt in enumerate(mts):
                        nc.tensor.matmul(
                            p_ps[:, i * (D + 1) : (i + 1) * (D + 1)],
                            xt[:, mt * P : (mt + 1) * P],
                            wproj[:, :],
                            start=True,
                            stop=True,
                        )
                    # s1 cols (stride D+1, offset D) -> s1a
                    nc.vector.tensor_copy(
                        s1a[:, mts[0] : mts[0] + w],
                        p_ps[:, D : w * (D + 1) : D + 1],
                    )
                    # F1 = exp(0.01 s1) needed now for the scaled copy
                    nc.scalar.activation(
                        F1a[:, mts[0] : mts[0] + w],
                        s1a[:, mts[0] : mts[0] + w],
                        Act.Exp,
                        scale=0.01,
                    )
                    # wxp[mt][:, :D] = F1 * Wx, alternating DVE/ACT
                    for i, mt in enumerate(mts):
                        dst = wxp[:, mt * (D + 1) : mt * (D + 1) + D]
                        src = p_ps[:, i * (D + 1) : i * (D + 1) + D]
                        sc = F1a[:, mt : mt + 1]
                        if mt % 2 == 0:
                            nc.vector.tensor_scalar(dst, src, sc, None, Alu.mult)
                        else:
                            nc.scalar.activation(dst, src, Act.Copy, scale=sc)

                # denominator column: wxp[mt][:, D] = F1[mt]  (strided dst)
                nc.vector.tensor_copy(
                    wxp[:, D : MT * (D + 1) : D + 1], F1a[:, :]
                )
                # t = exp(0.99 s1); negt; t in bf16
                nc.scalar.activation(t_a[:, :], s1a[:, :], Act.Exp, scale=0.99)
                nc.vector.tensor_scalar(nt_a[:, :], t_a[:, :], -1.0, None, Alu.mult)
                nc.vector.tensor_copy(tb_a[:, :], t_a[:, :])

                # Per-pair correction: ACT tiles are relu-form (missing the
                # t[src]-part), DVE tiles are max-form (complete).  For each
                # pair, c = sum over ACT-assigned mts of t[mt]^T @ wxp[mt].
                for pr in range(2):
                    amts = [
                        mt for mt in range(MT) if assign[pr * MT + mt] == "a"
                    ]
                    c_ps = vpool.tile(
                        [1, D + 1], f32, tag="cv", name=f"c_ps{pr}"
                    )
                    for k, mt in enumerate(amts):
                        nc.tensor.matmul(
                            c_ps[:, :],
                            tb_a[:, mt : mt + 1],
                            wxp[:, mt * (D + 1) : (mt + 1) * (D + 1)],
                            start=(k == 0),
                            stop=(k == len(amts) - 1),
                        )
                    nc.vector.tensor_copy(cvec_sb[pr][:, :], c_ps[:, :])

            # ---------------- main: 4 passes (one q each) ----------------
            def build_tile(idx):
                """double tile idx: pair p=idx//MT (q01 / q23), mt=idx%MT"""
                pair, mt = idx // MT, idx % MT
                e = assign[idx]
                dst = dt[idx][:, :]
                src = r_b[:, pair * 1024 : (pair + 1) * 1024]
                if e == "d":
                    # max-form (complete): max(t[src], r[dest]) — single op
                    nc.vector.tensor_scalar(
                        dst, src, t_a[:, mt : mt + 1], None, Alu.max
                    )
                else:
                    # relu-form (needs c correction): relu(r - t[src])
                    nc.scalar.activation(
                        dst, src, Act.Relu, bias=nt_a[:, mt : mt + 1]
                    )

            with tc.tile_pool(name="mpsum", bufs=2, space="PSUM") as mpool:
                def finalize_unit(q, acc, j):
                    ndc = q * 4 + j
                    rz = fpool.tile([P, 1], f32, tag="rz", name="rz")
                    nc.vector.reciprocal(rz[:, :], acc[j][:, D : D + 1])
                    lr = fpool.tile([P, D], f32, tag="lr", name="lr")
                    nc.scalar.activation(
                        lr[:, :], acc[j][:, :D], Act.Lrelu,
                        scale=rz[:, :], alpha=0.01,
                    )
                    o = opool.tile([P, D], f32, tag="o", name="o")
                    # last pass: DVE (idle by then); else GpSimd
                    sub_eng = nc.vector if q == 3 else nc.gpsimd
                    sub_eng.tensor_tensor(
                        o[:, :], xn_sb[:, ndc * D : (ndc + 1) * D],
                        lr[:, :], Alu.subtract,
                    )
                    eng = nc.sync if j % 2 == 0 else nc.scalar
                    eng.dma_start(out[:, ndc * D : (ndc + 1) * D], o[:, :])

                def finalize(q, acc):
                    for j in range(4):
                        finalize_unit(q, acc, j)

                cursor = [0]

                def ensure_built(upto):
                    while cursor[0] <= min(upto, 2 * MT - 1):
                        build_tile(cursor[0])
                        cursor[0] += 1

                pending = None
                for q in range(4):
                    pair = q // 2
                    off = (q % 2) * 512
                    acc = [
                        mpool.tile([P, D + 1], f32, tag=f"acc{j}", name=f"acc{j}")
                        for j in range(4)
                    ]
                    for mt in range(MT):
                        ti = pair * MT + mt
                        if q == 0:
                            ensure_built(mt + 6)       # stay ahead of the PE
                        elif q == 1:
                            ensure_built(MT + mt + 6)  # prefetch pair-1 tiles
                        else:
                            ensure_built(2 * MT - 1)
                        if mt == 6 and pending is not None:
                            finalize(*pending)
                            pending = None
                        for j in range(4):
                            nc.tensor.matmul(
                                acc[j][:, :],
                                dt[ti][:, off + j * P : off + (j + 1) * P],
                                wxp[:, mt * (D + 1) : (mt + 1) * (D + 1)],
                                start=(mt == 0),
                                stop=False,
                            )
                    # inject c (rank-1: ones x cvec), closes each group.
                    # On the last pass, finalize each unit right away.
                    for j in range(4):
                        nc.tensor.matmul(
                            acc[j][:, :], ones_1p[:, :], cvec_sb[pair][:, :],
                            start=False, stop=True,
                        )
                        if q == 3:
                            finalize_unit(q, acc, j)
                    pending = None if q == 3 else (q, acc)

    nc.compile()
    return nc


def _prep_inputs(x, W, a1, a2):
    """Per-core packed input. Core c: t = c//2, n-half h = c%2.

    xt is host-rotated so the core's own 2048 dest columns come first
    (a rotation does not change a sum over all source nodes).
    """
    import ml_dtypes

    bf16 = ml_dtypes.bfloat16
    x = np.asarray(x, dtype=np.float32)
    W = np.ascontiguousarray(np.asarray(W, dtype=np.float32))
    WT = np.ascontiguousarray(W.T)
    av = np.ascontiguousarray(
        np.stack([np.asarray(a1, np.float32), np.asarray(a2, np.float32)], axis=1)
    )
    prm = np.ascontiguousarray(
        np.concatenate([W, WT, av], axis=1).astype(bf16)
    )
    in_maps = []
    for c in range(8):
        t, h = c // 2, c % 2
        xt = x[:, t, :].T  # [D, N]
        if h == 1:
            xt = np.concatenate([xt[:, HALF:], xt[:, :HALF]], axis=1)
        xn = x[h * HALF : (h + 1) * HALF, t, :]  # [2048, 128]
        xn_packed = np.ascontiguousarray(
            xn.reshape(NT, P, D).transpose(1, 0, 2).reshape(P, NT * D)
        )
        in_maps.append(
            {
                "prm": prm,
                "xt": np.ascontiguousarray(xt.astype(bf16)),
                "xn": xn_packed,
            }
        )
    return in_maps


def _run(x, W, a1, a2, trace=False):
    from concourse.bass_utils import run_bass_kernel_spmd

    key = "nc"
    if key not in _CACHE:
        _CACHE[key] = _build()
    nc = _CACHE[key]
    in_maps = _prep_inputs(x, W, a1, a2)
    res = run_bass_kernel_spmd(nc, in_maps, list(range(8)), trace=trace)
    out_full = np.empty((N, T, D), dtype=np.float32)
    for c in range(8):
        t, h = c // 2, c % 2
        o = res.results[c]["out"].reshape(P, NT, D).transpose(1, 0, 2)
        out_full[h * HALF : (h + 1) * HALF, t, :] = o.reshape(HALF, D)
    return out_full, res


def kernel(x, W, a1, a2):
    out, _ = _run(x, W, a1, a2, trace=False)
    return out
